# revision 28
# baseline (speedup 1.0000x reference)
"""BlockSparseMLA Trainium2 kernel (v2 — dense-PE rewrite).

Sharding: 8 cores = 2 batches x 4 seq-quarters. Each core computes all 16
heads for its 512 queries. Host does block scoring / top-k, gathers
selected positions, builds the causal mask over selected keys, and patches
the all-masked rows (uniform attention) with a host-computed rank-1
fallback.

v2 changes vs v1:
 - bf16 operands everywhere except the q projection (x, w_q stay f32r);
   halves DMA traffic and enables the DVE 4x bf16 perf mode.
 - Stages D (q-proj + RoPE) and E (attention) are fused per head-pair
   chunk so the PE instruction stream stays dense (HAM stays warm).
 - Score matmuls row-packed: the two heads of a chunk run concurrently in
   disjoint 64-row groups of the PE array (K=64 each).
 - RoPE uses a host-permuted signed-sin table: qs = sinP*q (DVE bf16 4x),
   rot = P@qs (PE), qTr = cos*q + rot (one PSUM-side add). The separate
   sin multiply of the rotated copy is gone.
 - softmax denominator: no max-subtraction (scores are small), Z from the
   ones-matmul trick, 1/Z via reciprocal_approx_fast straight off PSUM.
   Z=0 rows (fully masked) produce garbage that the host overwrites.
 - Elementwise work spread across DVE / GpSimd / ACT so no engine exceeds
   the PE span.
"""

import sys

import numpy as np

sys.path.insert(0, "/opt/trn_rl_repo")

from contextlib import ExitStack

import concourse.bacc as bacc
import concourse.bass as bass
import concourse.mybir as mybir
import concourse.tile as tile

B, S, D = 2, 2048, 1024
H, HD, R = 16, 64, 128
BLOCK, TOPK = 64, 4
ROPE_BASE = 100000.0
SQ = S // 4
KEYS = TOPK * BLOCK  # 256
CK = D // 128  # c chunks (2 heads each)
DK = D // 128  # d chunks
F32 = mybir.dt.float32
BF16 = mybir.dt.bfloat16
NPBF16 = mybir.dt.np(BF16)


def _f32(a):
    return np.ascontiguousarray(a, dtype=np.float32)


def _bf16(a):
    return np.ascontiguousarray(np.asarray(a, dtype=np.float32).astype(NPBF16))


NPF8 = mybir.dt.np(mybir.dt.float8e4)
XSCALE, WQSCALE = 16.0, 512.0


def _f8(a, scale):
    a = np.asarray(a, dtype=np.float32) * scale
    return np.ascontiguousarray(np.clip(a, -448.0, 448.0).astype(NPF8))


def _wvup_zp(w_kv_up):
    """w_kv_up_v.T zero-padded so head h's 64 v-columns sit at
    cols h*128 + (h%2)*64 of a [R, 2048] matrix (other half zero)."""
    wv = np.asarray(w_kv_up, np.float32)[D:].T  # [R, D]
    wz = np.zeros((R, 2 * D), np.float32)
    for h in range(H):
        c0 = h * 128 + (h % 2) * 64
        wz[:, c0 : c0 + 64] = wv[:, h * 64 : (h + 1) * 64]
    return wz


def _perm_sigma():
    """sigma(p) = rotate-half source index (symmetric involution)."""
    p = np.arange(128)
    blk, e = p // 64, p % 64
    return blk * 64 + np.where(e < 32, e + 32, e - 32)


def _perm():
    """[128, 128] permutation matrix: (P @ v)[p] = v[sigma(p)]."""
    P = np.zeros((128, 128), np.float32)
    sig = _perm_sigma()
    P[sig, np.arange(128)] = 1.0
    return P


def _onesz():
    """[128, 256]: hi slice [hi*128:(hi+1)*128] has ones in its own
    64-row half (Z-broadcast matmul lhsT)."""
    oz = np.zeros((128, 256), np.float32)
    oz[:, 0:64] = 1.0
    oz[:, 192:256] = 1.0
    return oz


def host_prep(x, w_q, w_kv_down, w_kv_up, w_out, w_scorer):
    """Returns (in_maps for 8 cores, qmin[B], fb_rows[B, D])."""
    x = np.asarray(x, dtype=np.float32)
    nb = S // BLOCK

    reps = x.reshape(B, nb, BLOCK, D).mean(axis=2)
    scores = reps @ np.asarray(w_scorer, np.float32)[0]
    top = np.argsort(-scores, axis=1, kind="stable")[:, :TOPK]
    sel_blocks = np.sort(top, axis=1)
    qmin = sel_blocks[:, 0] * BLOCK
    sel_pos = (
        sel_blocks[:, :, None] * BLOCK + np.arange(BLOCK)[None, None, :]
    ).reshape(B, KEYS)

    # RoPE tables (fp32, mirrors reference._rope_tables)
    half = np.arange(0, HD, 2, dtype=np.float32) / np.float32(HD)
    inv_freq = np.float32(1.0) / np.power(np.float32(ROPE_BASE), half)
    freqs = np.arange(S, dtype=np.float32)[:, None] * inv_freq[None, :]
    emb = np.concatenate([freqs, freqs], axis=1)  # [S, HD]
    cos = np.cos(emb).astype(np.float32)
    sin = np.sin(emb).astype(np.float32)
    sgn = np.where(np.arange(HD) < HD // 2, np.float32(-1.0), np.float32(1.0))
    sins = sin * sgn[None, :]  # signed sin: rope(t) = t*cos + P(t)*sins

    # permuted signed-sin so sins*(P t) == P(sinsP * t)
    sig64 = _perm_sigma()[:64] % 64  # within the 64-dim head block
    sinsP = sins[:, sig64]  # [S, HD]

    # Fallback row for fully-masked queries
    latent_mean = x.mean(axis=1) @ np.asarray(w_kv_down, np.float32).T
    v_mean = latent_mean @ np.asarray(w_kv_up, np.float32)[D:].T
    fb_rows = v_mean @ np.asarray(w_out, np.float32).T

    w_q = np.asarray(w_q, np.float32)

    def pk(a, chunks):
        """[chunks*128, X] -> partition-major [128, chunks*X]."""
        a = np.asarray(a, np.float32)
        return a.reshape(chunks, 128, -1).transpose(1, 0, 2).reshape(128, -1)

    wq_pk = _bf16(pk(w_q.T, 8))                       # [128, 8*1024]
    wout_pk = _bf16(pk(np.asarray(w_out, np.float32).T, 8))  # [128, 8*1024]
    blobA_shared = [
        pk(np.asarray(w_kv_down, np.float32).T, 8),   # wkvd  [128, 1024]
        np.asarray(w_kv_up, np.float32)[:D].T,        # wkup  [128, 1024]
        _perm(),                                      # perm  [128, 128]
        np.eye(128, dtype=np.float32),                # ident [128, 128]
    ]
    blobB_shared = [
        _wvup_zp(w_kv_up),                            # wvup  [128, 2048]
        _onesz(),                                     # onesz [128, 256]
    ]
    in_maps = []
    for c in range(8):
        b, sq = divmod(c, 4)
        s0 = sq * SQ

        def pk(a, chunks):
            a = np.asarray(a, np.float32)
            return a.reshape(chunks, 128, -1).transpose(1, 0, 2).reshape(128, -1)

        blob0 = np.concatenate(
            [
                pk(x[b, sel_pos[b]].T, 8),            # xsel [128, 2048]
                blobA_shared[0],                      # wkvd [128, 1024]
            ],
            axis=1,
        )
        blobA = np.concatenate(
            blobA_shared[1:]
            + [
                np.tile(cos[sel_pos[b]].T, (2, 1)),   # cosk [128, 256]
                np.tile(sinsP[sel_pos[b]].T, (2, 1)), # sink [128, 256]
            ],
            axis=1,
        )
        blobB = np.concatenate(
            blobB_shared
            + [
                np.tile(cos[s0 : s0 + SQ].T, (2, 1)),    # cosq [128, 512]
                np.tile(sinsP[s0 : s0 + SQ].T, (2, 1)),  # sinq [128, 512]
                pk(
                    (sel_pos[b][:, None] <= (s0 + np.arange(SQ))[None, :]), 2
                ),                                        # mask [128, 1024]
            ],
            axis=1,
        )
        m = {
            "blob0": _bf16(blob0),
            "blobA": _bf16(blobA),
            "blobB": _bf16(blobB),
            "xTp": _bf16(pk(x[b, s0 : s0 + SQ].T, 8)),   # [128, 4096]
            "wqp": wq_pk,
            "woutp": wout_pk,
        }
        in_maps.append(m)
    return in_maps, qmin, fb_rows


def build_nc():
    nc = bacc.Bacc("TRN2", target_bir_lowering=False)

    # blob0: xsel 2048 | wkvd 1024
    Z_XSEL, Z_WKVD, Z_LEN = 0, 2048, 3072
    # blobA: wkup 1024 | perm 128 | ident 128 | cosk 256 | sink 256
    A_WKUP, A_PERM, A_ID, A_COSK, A_SINK = 0, 1024, 1152, 1280, 1536
    A_LEN = 1792
    blob0 = nc.dram_tensor("blob0", [128, Z_LEN], BF16, kind="ExternalInput")
    # blobB: wvup 2048 | onesz 256 | cosq 512 | sinq 512 | mask 1024
    B_WVUP, B_ONESZ, B_COSQ, B_SINQ, B_MASK = 0, 2048, 2304, 2816, 3328
    B_LEN = 4352
    blobA = nc.dram_tensor("blobA", [128, A_LEN], BF16, kind="ExternalInput")
    blobB = nc.dram_tensor("blobB", [128, B_LEN], BF16, kind="ExternalInput")
    xTp = nc.dram_tensor("xTp", [128, DK * SQ], BF16, kind="ExternalInput")
    wqp = nc.dram_tensor("wqp", [128, DK * D], BF16, kind="ExternalInput")
    woutp = nc.dram_tensor("woutp", [128, CK * D], BF16, kind="ExternalInput")
    out = nc.dram_tensor("out", [SQ, D], BF16, kind="ExternalOutput")

    EXP = mybir.ActivationFunctionType.Exp

    with tile.TileContext(nc) as tc, ExitStack() as ctx:
        const = ctx.enter_context(tc.tile_pool(name="const", bufs=1))

        # ---- persistent inputs: 5 big partition-major DMAs
        blob0_sb = const.tile([128, Z_LEN], BF16, tag="blob0")
        nc.sync.dma_start(blob0_sb[:], blob0[:, :])
        blobA_sb = const.tile([128, A_LEN], BF16, tag="blobA")
        nc.sync.dma_start(blobA_sb[:], blobA[:, :])
        blobB_sb = const.tile([128, B_LEN], BF16, tag="blobB")
        nc.sync.dma_start(blobB_sb[:], blobB[:, :])
        xT_sb = const.tile([128, DK, SQ], BF16, tag="xT")
        nc.sync.dma_start(
            xT_sb[:].rearrange("p k s -> p (k s)"), xTp[:, :]
        )
        wq_sb = const.tile([128, DK, D], BF16, tag="wq")
        nc.sync.dma_start(
            wq_sb[:].rearrange("p k s -> p (k s)"), wqp[:, :]
        )
        wout_sb = const.tile([128, CK, D], BF16, tag="wout")
        nc.sync.dma_start(
            wout_sb[:].rearrange("p k s -> p (k s)"), woutp[:, :]
        )

        xsel_sb = blob0_sb[:, Z_XSEL : Z_XSEL + 2048].rearrange(
            "p (k s) -> p k s", k=DK
        )
        wkvd_sb = blob0_sb[:, Z_WKVD : Z_WKVD + 1024].rearrange(
            "p (k r) -> p k r", k=DK
        )
        wkup_sb = blobA_sb[:, A_WKUP : A_WKUP + 1024]
        perm_sb = blobA_sb[:, A_PERM : A_PERM + 128]
        ident_sb = blobA_sb[:, A_ID : A_ID + 128]
        cosk_sb = blobA_sb[:, A_COSK : A_COSK + KEYS]
        sink_sb = blobA_sb[:, A_SINK : A_SINK + KEYS]
        wvup_sb = blobB_sb[:, B_WVUP : B_WVUP + 2 * D]
        onesz_sb = blobB_sb[:, B_ONESZ : B_ONESZ + 256]
        cosq_sb = blobB_sb[:, B_COSQ : B_COSQ + SQ]
        sinq_sb = blobB_sb[:, B_SINQ : B_SINQ + SQ]
        mask_sb = blobB_sb[:, B_MASK : B_MASK + 1024].rearrange(
            "p (m s) -> p m s", m=2
        )

        # ---- results that span stages
        kTr_sb = const.tile([128, CK, KEYS], BF16, tag="kTr")
        v_sb = const.tile([128, 2, 2 * D], BF16, tag="v")
        qTr_sb = const.tile([128, CK, SQ], BF16, tag="qTr")
        yT_sb = const.tile([128, CK, SQ], BF16, tag="yT")

        # ================= stages A-C: latent, kT+RoPE, v =================
        # One PSUM pool for the whole kernel: stages A-C and F borrow the
        # D+E tag slots, so no pool-close barrier (and no PE cold restart)
        # at stage boundaries.
        wk = ctx.enter_context(tc.tile_pool(name="wk_abc", bufs=2))
        ps = ctx.enter_context(tc.tile_pool(name="ps_all", bufs=1, space="PSUM"))
        if True:
            # PE warmup: junk matmuls on a memset tile (no DMA dependency,
            # so they are schedulable from t~0) promote the HAM clock gate
            # to 8/8 during the input-DMA lead-in. An explicit ordering
            # edge pins stage A behind them.
            wzero = wk.tile([128, 128], BF16, tag="wzero")
            nc.gpsimd.memset(wzero[:], 0.0)
            warm_ps = ps.tile([128, 128], F32, tag="z")
            warm_last = None
            for _ in range(80):
                warm_last = nc.tensor.matmul(
                    wzero_ps_view := warm_ps[:], wzero[:], wzero[:],
                    start=True, stop=True,
                )

            # A: latentT at selected positions [R, KEYS]
            lat_ps = ps.tile([128, KEYS], F32, tag="q", bufs=2)
            first_a = None
            for dk in range(DK):
                mm = nc.tensor.matmul(
                    lat_ps[:],
                    wkvd_sb[:, dk, :],
                    xsel_sb[:, dk, :],
                    start=(dk == 0),
                    stop=(dk == DK - 1),
                )
                if first_a is None:
                    first_a = mm
            bass._add_dep_helper(
                first_a.ins, warm_last.ins, sync=False,
                reason="stage A after PE warmup",
            )
            lat_sb = const.tile([128, KEYS], BF16, tag="lat")
            nc.scalar.copy(lat_sb[:], lat_ps[:])

            # B1: raw kT chunks (dense PE); two chunks share a PSUM bank
            # (second start=True only clears has_written bits, data
            # survives) so the evacuation count halves.
            k_praws = []
            for ckp in range(CK // 2):
                kp_ps = ps.tile(
                    [128, 2, SQ], F32, tag=("scA" if ckp % 2 == 0 else "scB"),
                    name=f"kp_ps{ckp}",
                )
                for j in range(2):
                    ck = 2 * ckp + j
                    nc.tensor.matmul(
                        kp_ps[:, 0, j * KEYS : (j + 1) * KEYS],
                        blobA_sb[:, A_WKUP + ck * 128 : A_WKUP + (ck + 1) * 128],
                        lat_sb[:],
                        start=True,
                        stop=True,
                    )
                k_raw2 = wk.tile(
                    [128, 2 * KEYS], BF16, tag=f"k_raw{ckp % 2}",
                    name=f"k_raw{ckp}", bufs=2,
                )
                if ckp % 2 == 0:
                    nc.scalar.copy(k_raw2[:], kp_ps[:, 0, :])
                else:
                    nc.vector.tensor_copy(k_raw2[:], kp_ps[:, 0, :])
                k_praws.append(k_raw2)

        # ============ stages D+E fused per head-pair chunk p ==============
        # Software-pipelined: the z/PV matmuls + normalization for chunk
        # p-1 are issued during iteration p so the exp/mask latency of
        # chunk p hides behind the q-projection of chunk p (the PE queue
        # is in-order; anything between scores(p) and qproj(p+1) that
        # waits on ACT/DVE stalls the array).
        with (
            tc.tile_pool(name="wk_de", bufs=2) as wkd,
            tc.tile_pool(name="exp_de", bufs=3) as wke,
        ):
            def stageC():
                # C: v [keys, c] zero-padded per head (dense PE); two nh
                # chunks per tile, one [128,1024] evacuation per pair.
                for mk in range(2):
                    for nhp in range(2):
                        vp_ps = ps.tile(
                            [128, 2, SQ], F32, tag=("scA" if nhp == 0 else "scB"),
                            name=f"v_ps{mk}_{nhp}",
                        )
                        for j in range(2):
                            nh = 2 * nhp + j
                            nc.tensor.matmul(
                                vp_ps[:, j, :],
                                lat_sb[:, mk * 128 : (mk + 1) * 128],
                                blobB_sb[:, B_WVUP + nh * 512 : B_WVUP + (nh + 1) * 512],
                                start=True,
                                stop=True,
                            )
                        if nhp == 0:
                            nc.scalar.copy(
                                v_sb[:, mk, 0:1024],
                                vp_ps[:].rearrange("p m s -> p (m s)"),
                            )
                        else:
                            nc.vector.tensor_copy(
                                v_sb[:, mk, 1024:2048],
                                vp_ps[:].rearrange("p m s -> p (m s)"),
                            )

            em_tiles = {}

            def blockK(p):
                k_raw = k_praws[p // 2][:, (p % 2) * KEYS : (p % 2 + 1) * KEYS]
                ks = wk.tile([128, KEYS], BF16, tag="ks")
                nc.vector.tensor_mul(ks[:], k_raw, sink_sb[:])
                kt1 = wk.tile([128, KEYS], BF16, tag="kt1")
                nc.vector.tensor_mul(kt1[:], k_raw, cosk_sb[:])
                k_rot = ps.tile([128, KEYS], F32, tag="q", bufs=2)
                nc.tensor.matmul(k_rot[:], perm_sb[:], ks[:], start=True, stop=True)
                nc.vector.tensor_add(kTr_sb[:, p, :], kt1[:], k_rot[:])

            def blockA1(p):
                # q chunk + RoPE
                q_ps = ps.tile([128, SQ], F32, tag="q", bufs=2)
                for dk in range(DK):
                    nc.tensor.matmul(
                        q_ps[:],
                        wq_sb[:, dk, p * 128 : (p + 1) * 128],
                        xT_sb[:, dk, :],
                        start=(dk == 0),
                        stop=(dk == DK - 1),
                    )
                q_raw = wkd.tile([128, SQ], BF16, tag="q_raw")
                nc.scalar.copy(q_raw[:], q_ps[:])
                qs = wkd.tile([128, SQ], BF16, tag="qs")
                nc.vector.tensor_mul(qs[:], q_raw[:], sinq_sb[:])
                qt1 = wkd.tile([128, SQ], BF16, tag="qt1")
                nc.vector.tensor_mul(qt1[:], q_raw[:], cosq_sb[:])
                q_rot = ps.tile([128, SQ], F32, tag="q", bufs=2)
                nc.tensor.matmul(q_rot[:], perm_sb[:], qs[:], start=True, stop=True)
                nc.vector.tensor_add(qTr_sb[:, p, :], qt1[:], q_rot[:])

            def blockA2(p):
                # scores for heads 2p, 2p+1 (row-packed pairs) + exp + mask
                scA = ps.tile([128, 2, SQ], F32, tag="scA")
                scB = ps.tile([128, 2, SQ], F32, tag="scB")
                for mk in range(2):
                    nc.tensor.matmul(
                        scA[:, mk, :],
                        kTr_sb[0:64, p, mk * 128 : (mk + 1) * 128],
                        qTr_sb[0:64, p, :],
                        start=True,
                        stop=True,
                    )
                    nc.tensor.matmul(
                        scB[:, mk, :],
                        kTr_sb[64:128, p, mk * 128 : (mk + 1) * 128],
                        qTr_sb[64:128, p, :],
                        start=True,
                        stop=True,
                    )
                expA = wke.tile([128, 2, SQ], BF16, tag="expA")
                nc.scalar.activation(
                    expA[:].rearrange("p m s -> p (m s)"),
                    scA[:].rearrange("p m s -> p (m s)"),
                    EXP,
                    scale=0.125,
                )
                expB = wke.tile([128, 2, SQ], BF16, tag="expB")
                nc.scalar.activation(
                    expB[:].rearrange("p m s -> p (m s)"),
                    scB[:].rearrange("p m s -> p (m s)"),
                    EXP,
                    scale=0.125,
                )
                emA = wke.tile([128, 2, SQ], BF16, tag="emA")
                nc.vector.tensor_mul(
                    emA[:].rearrange("p m s -> p (m s)"),
                    expA[:].rearrange("p m s -> p (m s)"),
                    mask_sb[:].rearrange("p m s -> p (m s)"),
                )
                emB = wke.tile([128, 2, SQ], BF16, tag="emB")
                nc.vector.tensor_mul(
                    emB[:].rearrange("p m s -> p (m s)"),
                    expB[:].rearrange("p m s -> p (m s)"),
                    mask_sb[:].rearrange("p m s -> p (m s)"),
                )
                em_tiles[p] = (emA, emB)

            def blockB(p):
                emA, emB = em_tiles.pop(p)
                z_ps = ps.tile([128, SQ], F32, tag="z")
                pv_ps = ps.tile([128, SQ], F32, tag="pv")
                for hi in range(2):
                    em = emA if hi == 0 else emB
                    h = 2 * p + hi
                    for mk in range(2):
                        nc.tensor.matmul(
                            z_ps[:],
                            blobB_sb[:, B_ONESZ + hi * 128 : B_ONESZ + (hi + 1) * 128],
                            em[:, mk, :],
                            start=(hi == 0 and mk == 0),
                            stop=(hi == 1 and mk == 1),
                        )
                        nc.tensor.matmul(
                            pv_ps[:],
                            v_sb[:, mk, h * 128 : (h + 1) * 128],
                            em[:, mk, :],
                            start=(hi == 0 and mk == 0),
                            stop=(hi == 1 and mk == 1),
                        )
                zr = wkd.tile([128, SQ], F32, tag="zr")
                nc.vector.reciprocal_approx_fast(zr[:], z_ps[:])
                nc.vector.tensor_mul(yT_sb[:, p, :], pv_ps[:], zr[:])

            for p in range(CK):
                blockK(p)
                blockA1(p)
                if p == 0:
                    stageC()
                if p > 0:
                    blockB(p - 1)
                blockA2(p)
            blockB(CK - 1)

        # ================= stage F: out = yT.T @ woutT ====================
        with (
            tc.tile_pool(name="ost", bufs=4) as ost,
        ):
            fA = ps.tile([128, 2, SQ], F32, tag="scA", name="fA")
            fB = ps.tile([128, 2, SQ], F32, tag="scB", name="fB")
            fq0 = ps.tile([128, SQ], F32, tag="q", bufs=2, name="fq0")
            fq1 = ps.tile([128, SQ], F32, tag="q", bufs=2, name="fq1")
            fz = ps.tile([128, SQ], F32, tag="z", name="fz")
            fp = ps.tile([128, SQ], F32, tag="pv", name="fp")
            outps = [
                fA[:, 0, :], fA[:, 1, :], fB[:, 0, :], fB[:, 1, :],
                fq0[:], fq1[:], fz[:], fp[:],
            ]
            for ck in range(CK):
                for st in range(4):
                    for dh in range(2):
                        nc.tensor.matmul(
                            outps[st * 2 + dh],
                            yT_sb[:, ck, st * 128 : (st + 1) * 128],
                            wout_sb[:, ck, dh * 512 : (dh + 1) * 512],
                            start=(ck == 0),
                            stop=(ck == CK - 1),
                        )
            for st in range(4):
                o_sb = ost.tile([128, D], BF16, tag="osb")
                nc.scalar.copy(o_sb[:, 0:512], outps[st * 2])
                nc.vector.tensor_copy(o_sb[:, 512:1024], outps[st * 2 + 1])
                nc.sync.dma_start(out[st * 128 : (st + 1) * 128, :], o_sb[:])

    nc.compile()
    return nc


_NC_CACHE = {}


def _get_nc():
    if "nc" not in _NC_CACHE:
        _NC_CACHE["nc"] = build_nc()
    return _NC_CACHE["nc"]


TRACE = False  # set by test harness to capture an NTFF profile
LAST_RESULTS = None


def kernel(x, w_q, w_kv_down, w_kv_up, w_out, w_scorer):
    global LAST_RESULTS
    from concourse.bass_utils import run_bass_kernel_spmd

    in_maps, qmin, fb_rows = host_prep(x, w_q, w_kv_down, w_kv_up, w_out, w_scorer)
    nc = _get_nc()
    res = run_bass_kernel_spmd(nc, in_maps, core_ids=list(range(8)), trace=TRACE)
    LAST_RESULTS = res
    out = np.empty((B, S, D), np.float32)
    for c in range(8):
        b, sq = divmod(c, 4)
        out[b, sq * SQ : (sq + 1) * SQ] = np.asarray(
            res.results[c]["out"], dtype=np.float32
        )
    for b in range(B):
        if qmin[b] > 0:
            out[b, : qmin[b]] = fb_rows[b]
    return out
